# revision 4
# baseline (speedup 1.0000x reference)
"""DBRX-style MoE (E=16, top-4, C=2048, H=3584, N=1024 tokens) on 8 TRN2 cores.

Strategy (expert-parallel x H-tensor-parallel, routed):
  - Host gating in fp64 (logits -> top-4 -> softmax weights), as before.
  - Experts are sorted by token count into 4 rank-blocks of 4; group g
    (one expert per block) is owned by core PAIR (2g, 2g+1). Each core of
    the pair computes HALF of the FFN hidden dim (14 of 28 h-chunks) for
    all 4 experts of its group:
      uT/gT = Wup_half/Wg_half @ xT        (PSUM over C chunks)
      hT    = silu(gT) * uT * gate_weight  (elementwise in its H half)
      yT_p  = Wdown[:, half] @ hT          (partial over its 14 h-chunks)
  - Host adds the pair's partial yT (fp16) and scatter-adds to token rows.
  - Per-slot caps are the rank-block maxima (padded to 4), identical on
    all cores (SPMD single program); per-core padded visits ~1068 vs the
    ~1112 of the 2-experts-per-core layout, and weight DMA bytes are
    unchanged (each weight byte still read once chip-wide).
  - Head: first (smallest) expert's xg DMA is split in 4 and interleaved
    with wu0/wg0 so the PE starts streaming at ~2us.
"""

import math

import numpy as np

E, TOPK = 16, 4
C, H = 2048, 3584
B, T = 2, 512
N = B * T
N_CORES = 8
N_GROUPS = 4
SLOTS = 4  # experts per core (= rank blocks)
C_CHUNKS = C // 128  # 16
H_CHUNKS = H // 128  # 28
HL = H_CHUNKS // 2  # 14 h-chunks per core (half of the FFN dim)

_NC_CACHE: dict[tuple, object] = {}


def _token_tiles(cap: int) -> list[tuple[int, int]]:
    tiles = []
    off = 0
    while off < cap:
        sz = min(512, cap - off)
        tiles.append((off, sz))
        off += sz
    return tiles


def _build_nc(caps: tuple):
    import concourse.bacc as bacc
    import concourse.mybir as mybir
    import concourse.tile as tile

    f32 = mybir.dt.float32
    f16 = mybir.dt.float16

    nc = bacc.Bacc("TRN2", target_bir_lowering=False, debug=False)
    xgs = [
        nc.dram_tensor(f"xg{j}", [128, C_CHUNKS * caps[j]], f16, kind="ExternalInput")
        for j in range(SLOTS)
    ]
    wbs = [
        nc.dram_tensor(f"wb{j}", [128, caps[j]], f32, kind="ExternalInput")
        for j in range(SLOTS)
    ]
    wug = nc.dram_tensor(
        "wug", [SLOTS, HL, 2, 128, C_CHUNKS * 128], f16, kind="ExternalInput"
    )
    wd = nc.dram_tensor(
        "wd", [SLOTS, C_CHUNKS, 128, HL * 128], f16, kind="ExternalInput"
    )
    yts = [
        nc.dram_tensor(f"yt{j}", [C_CHUNKS, 128, caps[j]], f16, kind="ExternalOutput")
        for j in range(SLOTS)
    ]

    with tile.TileContext(nc) as tc:
        with (
            tc.tile_pool(name="xp", bufs=2) as xp,
            tc.tile_pool(name="wp", bufs=8) as wp,
            tc.tile_pool(name="hp", bufs=2) as hp,
            tc.tile_pool(name="wdp", bufs=4) as wdp,
            tc.tile_pool(name="sp", bufs=3) as sp,
            tc.tile_pool(name="psu", bufs=2, space="PSUM") as psu,
            tc.tile_pool(name="psg", bufs=2, space="PSUM") as psg,
            tc.tile_pool(name="psy", bufs=2, space="PSUM") as psy,
        ):
            # Smallest expert first: the DMA-gated warmup covers fewer bytes.
            expert_order = sorted(range(SLOTS), key=lambda j: caps[j])
            for ei, e in enumerate(expert_order):
                cap = caps[e]
                tts = _token_tiles(cap)
                xt = xp.tile([128, C_CHUNKS * cap], f16, tag="xg")
                wbt = xp.tile([128, cap], f32, tag="wb")
                head_w = {}
                if ei == 0:
                    qc = C_CHUNKS // 4
                    nc.sync.dma_start(xt[:, : qc * cap], xgs[e].ap()[:, : qc * cap])
                    wu0 = wp.tile([128, C_CHUNKS * 128], f16, tag="wug")
                    nc.sync.dma_start(wu0[:], wug.ap()[e, 0, 0])
                    nc.sync.dma_start(
                        xt[:, qc * cap : 2 * qc * cap],
                        xgs[e].ap()[:, qc * cap : 2 * qc * cap],
                    )
                    wg0 = wp.tile([128, C_CHUNKS * 128], f16, tag="wug")
                    nc.sync.dma_start(wg0[:], wug.ap()[e, 0, 1])
                    nc.sync.dma_start(
                        xt[:, 2 * qc * cap : 3 * qc * cap],
                        xgs[e].ap()[:, 2 * qc * cap : 3 * qc * cap],
                    )
                    nc.sync.dma_start(wbt[:], wbs[e].ap())
                    nc.sync.dma_start(
                        xt[:, 3 * qc * cap :], xgs[e].ap()[:, 3 * qc * cap :]
                    )
                    head_w = {0: (wu0, wg0)}
                else:
                    nc.sync.dma_start(xt[:], xgs[e].ap())
                    nc.sync.dma_start(wbt[:], wbs[e].ap())
                ht = hp.tile([128, HL * cap], f16, tag="ht")

                for h in range(HL):
                    if h in head_w:
                        wu, wg = head_w[h]
                    else:
                        wu = wp.tile([128, C_CHUNKS * 128], f16, tag="wug")
                        nc.sync.dma_start(wu[:], wug.ap()[e, h, 0])
                        wg = wp.tile([128, C_CHUNKS * 128], f16, tag="wug")
                        nc.sync.dma_start(wg[:], wug.ap()[e, h, 1])
                    for off, sz in tts:
                        ups = psu.tile([128, sz], f32, tag="u")
                        gps = psg.tile([128, sz], f32, tag="g")
                        for c in range(C_CHUNKS):
                            nc.tensor.matmul(
                                ups[:],
                                wu[:, c * 128 : (c + 1) * 128],
                                xt[:, c * cap + off : c * cap + off + sz],
                                start=(c == 0),
                                stop=(c == C_CHUNKS - 1),
                            )
                        for c in range(C_CHUNKS):
                            nc.tensor.matmul(
                                gps[:],
                                wg[:, c * 128 : (c + 1) * 128],
                                xt[:, c * cap + off : c * cap + off + sz],
                                start=(c == 0),
                                stop=(c == C_CHUNKS - 1),
                            )
                        sg = sp.tile([128, cap], f32, tag="sg")
                        nc.scalar.activation(
                            sg[:, :sz], gps[:], mybir.ActivationFunctionType.Silu
                        )
                        uw = sp.tile([128, cap], f32, tag="uw")
                        nc.vector.tensor_mul(
                            uw[:, :sz], ups[:], wbt[:, off : off + sz]
                        )
                        nc.vector.tensor_mul(
                            ht[:, h * cap + off : h * cap + off + sz],
                            sg[:, :sz],
                            uw[:, :sz],
                        )

                for ct in range(C_CHUNKS):
                    wdt = wdp.tile([128, HL * 128], f16, tag="wd")
                    nc.sync.dma_start(wdt[:], wd.ap()[e, ct])
                    for off, sz in tts:
                        yps = psy.tile([128, sz], f32, tag="y")
                        for h in range(HL):
                            nc.tensor.matmul(
                                yps[:],
                                wdt[:, h * 128 : (h + 1) * 128],
                                ht[:, h * cap + off : h * cap + off + sz],
                                start=(h == 0),
                                stop=(h == HL - 1),
                            )
                        yo = sp.tile([128, cap], f16, tag="yo")
                        nc.vector.tensor_copy(yo[:, :sz], yps[:])
                        nc.sync.dma_start(yts[e].ap()[ct, :, off : off + sz], yo[:, :sz])
    nc.compile()
    return nc


def _get_nc(caps: tuple):
    if caps not in _NC_CACHE:
        _NC_CACHE[caps] = _build_nc(caps)
    return _NC_CACHE[caps]


def _route(xf: np.ndarray, gate_inp: np.ndarray):
    logits = xf.astype(np.float64) @ gate_inp.astype(np.float64).T  # [N, E]
    topi = np.argsort(-logits, axis=1, kind="stable")[:, :TOPK]  # [N, K]
    topv = np.take_along_axis(logits, topi, axis=1)
    w = np.exp(topv - topv[:, :1])
    w /= w.sum(axis=1, keepdims=True)
    idxs, wts = [], []
    for e in range(E):
        sel = topi == e
        rows = np.nonzero(sel.any(axis=1))[0]
        k_of_row = np.argmax(sel[rows], axis=1)
        idxs.append(rows.astype(np.int64))
        wts.append(w[rows, k_of_row])
    return idxs, wts


def kernel(x, W_up, W_gate, W_down, gate_inp):
    from concourse import bass_utils

    x = np.ascontiguousarray(np.asarray(x, dtype=np.float32))
    W_up = np.asarray(W_up, dtype=np.float32)
    W_gate = np.asarray(W_gate, dtype=np.float32)
    W_down = np.asarray(W_down, dtype=np.float32)
    gate_inp = np.asarray(gate_inp, dtype=np.float32)

    xf = x.reshape(N, C)
    idxs, wts = _route(xf, gate_inp)
    counts = np.array([len(i) for i in idxs])
    # rank-block assignment: block j = experts ranked [4j, 4j+4) by count;
    # group g (core pair 2g, 2g+1) takes one expert from each block.
    order = np.argsort(-counts, kind="stable")
    assign = [
        [int(order[j * N_GROUPS + g]) for j in range(SLOTS)] for g in range(N_GROUPS)
    ]
    caps = tuple(
        max(64, int(math.ceil(counts[order[j * N_GROUPS]] / 4)) * 4)
        for j in range(SLOTS)
    )

    # Per-group (shared by the core pair): gathered x, combine weights.
    xg_g, wb_g = [], []
    for g in range(N_GROUPS):
        xgl, wbl = [], []
        for j in range(SLOTS):
            cap = caps[j]
            eidx = assign[g][j]
            idx, wvec = idxs[eidx], wts[eidx]
            cnt = len(idx)
            xge = np.zeros((cap, C), np.float16)
            xge[:cnt] = xf[idx]
            xgl.append(
                xge.reshape(cap, C_CHUNKS, 128)
                .transpose(2, 1, 0)
                .reshape(128, C_CHUNKS * cap)
            )
            wbe = np.zeros((128, cap), np.float32)
            wbe[:, :cnt] = np.float32(wvec)[None, :]
            wbl.append(wbe)
        xg_g.append(xgl)
        wb_g.append(wbl)

    in_maps = []
    for core in range(N_CORES):
        g, half = core // 2, core % 2
        hs, he = half * HL, (half + 1) * HL
        wug = np.empty((SLOTS, HL, 2, 128, C_CHUNKS * 128), np.float16)
        wd = np.empty((SLOTS, C_CHUNKS, 128, HL * 128), np.float16)
        for j in range(SLOTS):
            eidx = assign[g][j]
            # stationary up/gate tiles for this core's h-chunks
            wug[j, :, 0] = (
                W_up[eidx]
                .reshape(H_CHUNKS, 128, C_CHUNKS, 128)[hs:he]
                .transpose(0, 3, 2, 1)
                .reshape(HL, 128, C_CHUNKS * 128)
            )
            wug[j, :, 1] = (
                W_gate[eidx]
                .reshape(H_CHUNKS, 128, C_CHUNKS, 128)[hs:he]
                .transpose(0, 3, 2, 1)
                .reshape(HL, 128, C_CHUNKS * 128)
            )
            # down-proj tiles restricted to this core's h-chunks
            wd[j] = (
                W_down[eidx]
                .reshape(C_CHUNKS, 128, H_CHUNKS, 128)[:, :, hs:he]
                .transpose(0, 3, 2, 1)
                .reshape(C_CHUNKS, 128, HL * 128)
            )
        im = {"wug": wug, "wd": wd}
        for j in range(SLOTS):
            im[f"xg{j}"] = xg_g[g][j]
            im[f"wb{j}"] = wb_g[g][j]
        in_maps.append(im)

    nc = _get_nc(caps)
    res = bass_utils.run_bass_kernel_spmd(nc, in_maps, core_ids=list(range(N_CORES)))
    kernel.last_result = res

    y = np.zeros((N, C), np.float32)
    for g in range(N_GROUPS):
        for j in range(SLOTS):
            eidx = assign[g][j]
            idx = idxs[eidx]
            cnt = len(idx)
            ytf = (
                res.results[2 * g][f"yt{j}"].astype(np.float32)
                + res.results[2 * g + 1][f"yt{j}"].astype(np.float32)
            ).reshape(C, caps[j])
            y[idx] += ytf[:, :cnt].T
    return y.reshape(B, T, C)


# revision 5
# speedup vs baseline: 1.0070x; 1.0070x over previous
"""DBRX-style MoE (E=16, top-4, C=2048, H=3584, N=1024 tokens) on 8 TRN2 cores.

Strategy (expert-parallel x H-tensor-parallel, routed):
  - Host gating in fp64 (logits -> top-4 -> softmax weights), as before.
  - Experts are sorted by token count into 4 rank-blocks of 4; group g
    (one expert per block) is owned by core PAIR (2g, 2g+1). Each core of
    the pair computes HALF of the FFN hidden dim (14 of 28 h-chunks) for
    all 4 experts of its group:
      uT/gT = Wup_half/Wg_half @ xT        (PSUM over C chunks)
      hT    = silu(gT) * uT * gate_weight  (elementwise in its H half)
      yT_p  = Wdown[:, half] @ hT          (partial over its 14 h-chunks)
  - Host adds the pair's partial yT (fp16) and scatter-adds to token rows.
  - Per-slot caps are the rank-block maxima (padded to 4), identical on
    all cores (SPMD single program); per-core padded visits ~1068 vs the
    ~1112 of the 2-experts-per-core layout, and weight DMA bytes are
    unchanged (each weight byte still read once chip-wide).
  - Head: first (smallest) expert's xg DMA is split in 4 and interleaved
    with wu0/wg0 so the PE starts streaming at ~2us.
"""

import math

import numpy as np

E, TOPK = 16, 4
C, H = 2048, 3584
B, T = 2, 512
N = B * T
N_CORES = 8
N_GROUPS = 4
SLOTS = 4  # experts per core (= rank blocks)
C_CHUNKS = C // 128  # 16
H_CHUNKS = H // 128  # 28
HL = H_CHUNKS // 2  # 14 h-chunks per core (half of the FFN dim)

_NC_CACHE: dict[tuple, object] = {}


def _token_tiles(cap: int) -> list[tuple[int, int]]:
    tiles = []
    off = 0
    while off < cap:
        sz = min(512, cap - off)
        tiles.append((off, sz))
        off += sz
    return tiles


def _build_nc(caps: tuple):
    import concourse.bacc as bacc
    import concourse.mybir as mybir
    import concourse.tile as tile

    f32 = mybir.dt.float32
    f16 = mybir.dt.float16

    nc = bacc.Bacc("TRN2", target_bir_lowering=False, debug=False)
    xgs = [
        nc.dram_tensor(f"xg{j}", [128, C_CHUNKS * caps[j]], f16, kind="ExternalInput")
        for j in range(SLOTS)
    ]
    wbs = [
        nc.dram_tensor(f"wb{j}", [128, caps[j]], f32, kind="ExternalInput")
        for j in range(SLOTS)
    ]
    wug = nc.dram_tensor(
        "wug", [SLOTS, HL, 2, 128, C_CHUNKS * 128], f16, kind="ExternalInput"
    )
    wd = nc.dram_tensor(
        "wd", [SLOTS, C_CHUNKS, 128, HL * 128], f16, kind="ExternalInput"
    )
    yts = [
        nc.dram_tensor(f"yt{j}", [C_CHUNKS, 128, caps[j]], f16, kind="ExternalOutput")
        for j in range(SLOTS)
    ]

    with tile.TileContext(nc) as tc:
        with (
            tc.tile_pool(name="xp", bufs=2) as xp,
            tc.tile_pool(name="wp", bufs=12) as wp,
            tc.tile_pool(name="hp", bufs=2) as hp,
            tc.tile_pool(name="wdp", bufs=12) as wdp,
            tc.tile_pool(name="sp", bufs=3) as sp,
            tc.tile_pool(name="psu", bufs=2, space="PSUM") as psu,
            tc.tile_pool(name="psg", bufs=2, space="PSUM") as psg,
            tc.tile_pool(name="psy", bufs=2, space="PSUM") as psy,
        ):
            # Smallest expert first: the DMA-gated warmup covers fewer bytes.
            expert_order = sorted(range(SLOTS), key=lambda j: caps[j])
            for ei, e in enumerate(expert_order):
                cap = caps[e]
                tts = _token_tiles(cap)
                xt = xp.tile([128, C_CHUNKS * cap], f16, tag="xg")
                wbt = xp.tile([128, cap], f32, tag="wb")
                head_w = {}
                if ei == 0:
                    qc = C_CHUNKS // 4
                    nc.sync.dma_start(xt[:, : qc * cap], xgs[e].ap()[:, : qc * cap])
                    wu0 = wp.tile([128, C_CHUNKS * 128], f16, tag="wug")
                    nc.sync.dma_start(wu0[:], wug.ap()[e, 0, 0])
                    nc.sync.dma_start(
                        xt[:, qc * cap : 2 * qc * cap],
                        xgs[e].ap()[:, qc * cap : 2 * qc * cap],
                    )
                    wg0 = wp.tile([128, C_CHUNKS * 128], f16, tag="wug")
                    nc.sync.dma_start(wg0[:], wug.ap()[e, 0, 1])
                    nc.sync.dma_start(
                        xt[:, 2 * qc * cap : 3 * qc * cap],
                        xgs[e].ap()[:, 2 * qc * cap : 3 * qc * cap],
                    )
                    nc.sync.dma_start(wbt[:], wbs[e].ap())
                    nc.sync.dma_start(
                        xt[:, 3 * qc * cap :], xgs[e].ap()[:, 3 * qc * cap :]
                    )
                    head_w = {0: (wu0, wg0)}
                else:
                    nc.sync.dma_start(xt[:], xgs[e].ap())
                    nc.sync.dma_start(wbt[:], wbs[e].ap())
                ht = hp.tile([128, HL * cap], f16, tag="ht")

                for h in range(HL):
                    if h in head_w:
                        wu, wg = head_w[h]
                    else:
                        wu = wp.tile([128, C_CHUNKS * 128], f16, tag="wug")
                        nc.sync.dma_start(wu[:], wug.ap()[e, h, 0])
                        wg = wp.tile([128, C_CHUNKS * 128], f16, tag="wug")
                        nc.sync.dma_start(wg[:], wug.ap()[e, h, 1])
                    for off, sz in tts:
                        ups = psu.tile([128, sz], f32, tag="u")
                        gps = psg.tile([128, sz], f32, tag="g")
                        for c in range(C_CHUNKS):
                            nc.tensor.matmul(
                                ups[:],
                                wu[:, c * 128 : (c + 1) * 128],
                                xt[:, c * cap + off : c * cap + off + sz],
                                start=(c == 0),
                                stop=(c == C_CHUNKS - 1),
                            )
                        for c in range(C_CHUNKS):
                            nc.tensor.matmul(
                                gps[:],
                                wg[:, c * 128 : (c + 1) * 128],
                                xt[:, c * cap + off : c * cap + off + sz],
                                start=(c == 0),
                                stop=(c == C_CHUNKS - 1),
                            )
                        sg = sp.tile([128, cap], f32, tag="sg")
                        nc.scalar.activation(
                            sg[:, :sz], gps[:], mybir.ActivationFunctionType.Silu
                        )
                        uw = sp.tile([128, cap], f32, tag="uw")
                        nc.vector.tensor_mul(
                            uw[:, :sz], ups[:], wbt[:, off : off + sz]
                        )
                        nc.vector.tensor_mul(
                            ht[:, h * cap + off : h * cap + off + sz],
                            sg[:, :sz],
                            uw[:, :sz],
                        )

                for ct in range(C_CHUNKS):
                    wdt = wdp.tile([128, HL * 128], f16, tag="wd")
                    nc.sync.dma_start(wdt[:], wd.ap()[e, ct])
                    for off, sz in tts:
                        yps = psy.tile([128, sz], f32, tag="y")
                        for h in range(HL):
                            nc.tensor.matmul(
                                yps[:],
                                wdt[:, h * 128 : (h + 1) * 128],
                                ht[:, h * cap + off : h * cap + off + sz],
                                start=(h == 0),
                                stop=(h == HL - 1),
                            )
                        yo = sp.tile([128, cap], f16, tag="yo")
                        nc.vector.tensor_copy(yo[:, :sz], yps[:])
                        nc.sync.dma_start(yts[e].ap()[ct, :, off : off + sz], yo[:, :sz])
    nc.compile()
    return nc


def _get_nc(caps: tuple):
    if caps not in _NC_CACHE:
        _NC_CACHE[caps] = _build_nc(caps)
    return _NC_CACHE[caps]


def _route(xf: np.ndarray, gate_inp: np.ndarray):
    logits = xf.astype(np.float64) @ gate_inp.astype(np.float64).T  # [N, E]
    topi = np.argsort(-logits, axis=1, kind="stable")[:, :TOPK]  # [N, K]
    topv = np.take_along_axis(logits, topi, axis=1)
    w = np.exp(topv - topv[:, :1])
    w /= w.sum(axis=1, keepdims=True)
    idxs, wts = [], []
    for e in range(E):
        sel = topi == e
        rows = np.nonzero(sel.any(axis=1))[0]
        k_of_row = np.argmax(sel[rows], axis=1)
        idxs.append(rows.astype(np.int64))
        wts.append(w[rows, k_of_row])
    return idxs, wts


def kernel(x, W_up, W_gate, W_down, gate_inp):
    from concourse import bass_utils

    x = np.ascontiguousarray(np.asarray(x, dtype=np.float32))
    W_up = np.asarray(W_up, dtype=np.float32)
    W_gate = np.asarray(W_gate, dtype=np.float32)
    W_down = np.asarray(W_down, dtype=np.float32)
    gate_inp = np.asarray(gate_inp, dtype=np.float32)

    xf = x.reshape(N, C)
    idxs, wts = _route(xf, gate_inp)
    counts = np.array([len(i) for i in idxs])
    # rank-block assignment: block j = experts ranked [4j, 4j+4) by count;
    # group g (core pair 2g, 2g+1) takes one expert from each block.
    order = np.argsort(-counts, kind="stable")
    assign = [
        [int(order[j * N_GROUPS + g]) for j in range(SLOTS)] for g in range(N_GROUPS)
    ]
    caps = tuple(
        max(64, int(math.ceil(counts[order[j * N_GROUPS]] / 4)) * 4)
        for j in range(SLOTS)
    )

    # Per-group (shared by the core pair): gathered x, combine weights.
    xg_g, wb_g = [], []
    for g in range(N_GROUPS):
        xgl, wbl = [], []
        for j in range(SLOTS):
            cap = caps[j]
            eidx = assign[g][j]
            idx, wvec = idxs[eidx], wts[eidx]
            cnt = len(idx)
            xge = np.zeros((cap, C), np.float16)
            xge[:cnt] = xf[idx]
            xgl.append(
                xge.reshape(cap, C_CHUNKS, 128)
                .transpose(2, 1, 0)
                .reshape(128, C_CHUNKS * cap)
            )
            wbe = np.zeros((128, cap), np.float32)
            wbe[:, :cnt] = np.float32(wvec)[None, :]
            wbl.append(wbe)
        xg_g.append(xgl)
        wb_g.append(wbl)

    in_maps = []
    for core in range(N_CORES):
        g, half = core // 2, core % 2
        hs, he = half * HL, (half + 1) * HL
        wug = np.empty((SLOTS, HL, 2, 128, C_CHUNKS * 128), np.float16)
        wd = np.empty((SLOTS, C_CHUNKS, 128, HL * 128), np.float16)
        for j in range(SLOTS):
            eidx = assign[g][j]
            # stationary up/gate tiles for this core's h-chunks
            wug[j, :, 0] = (
                W_up[eidx]
                .reshape(H_CHUNKS, 128, C_CHUNKS, 128)[hs:he]
                .transpose(0, 3, 2, 1)
                .reshape(HL, 128, C_CHUNKS * 128)
            )
            wug[j, :, 1] = (
                W_gate[eidx]
                .reshape(H_CHUNKS, 128, C_CHUNKS, 128)[hs:he]
                .transpose(0, 3, 2, 1)
                .reshape(HL, 128, C_CHUNKS * 128)
            )
            # down-proj tiles restricted to this core's h-chunks
            wd[j] = (
                W_down[eidx]
                .reshape(C_CHUNKS, 128, H_CHUNKS, 128)[:, :, hs:he]
                .transpose(0, 3, 2, 1)
                .reshape(C_CHUNKS, 128, HL * 128)
            )
        im = {"wug": wug, "wd": wd}
        for j in range(SLOTS):
            im[f"xg{j}"] = xg_g[g][j]
            im[f"wb{j}"] = wb_g[g][j]
        in_maps.append(im)

    nc = _get_nc(caps)
    res = bass_utils.run_bass_kernel_spmd(nc, in_maps, core_ids=list(range(N_CORES)))
    kernel.last_result = res

    y = np.zeros((N, C), np.float32)
    for g in range(N_GROUPS):
        for j in range(SLOTS):
            eidx = assign[g][j]
            idx = idxs[eidx]
            cnt = len(idx)
            ytf = (
                res.results[2 * g][f"yt{j}"].astype(np.float32)
                + res.results[2 * g + 1][f"yt{j}"].astype(np.float32)
            ).reshape(C, caps[j])
            y[idx] += ytf[:, :cnt].T
    return y.reshape(B, T, C)
